# revision 1
# baseline (speedup 1.0000x reference)
"""GroupedExpertNetwork (SwiGLU per-expert MLP) Trainium2 kernel.

Expert-parallel: 8 experts -> 8 NeuronCores, one expert per core.
Per core:  g = x @ gate; u = x @ inner; h = silu(g)*u; out = h @ outp
Shapes per expert: x [T=2048, D=2048], gate/inner [D, I=4096], outp [I, D].

Structure: paired T blocks — each weight chunk is streamed once per pair of
T blocks, halving input DMA to 126 MB/core (vs 226 MB when re-streaming per
block). Per T block the kernel computes hT [I, TB] fully resident in SBUF
(bf16), then the output matmuls accumulate over the full I=4096 in PSUM.
All matmul free dims are 512 (one PSUM bank), contraction 128 per
instruction, bf16 operands, f32 accumulation in a single shared 8-bank
PSUM pool (deeper rotation = fewer PE stalls on bank reuse); output is f32.
"""

import numpy as np
import ml_dtypes

E, T, D, I = 8, 2048, 2048, 4096
P = 128
TB = 512                 # T block size
NT = T // TB             # 4 T blocks
NPAIR = NT // 2          # 2 pairs
IC = 256                 # I chunk for gate/inner weight streaming
NIC = I // IC            # 16
DC = 256                 # D chunk for output weight streaming
NDC = D // DC            # 8
KD = D // P              # 16 contraction chunks for gate/inner matmuls
KI = I // P              # 32 contraction chunks for output matmul

_COMPILED = None


def _build_program(reps=1, qsplit=False, psum_shared=False, deep_bufs=False):
    import concourse.mybir as mybir
    import concourse.tile as tile
    from concourse import bacc

    bf16 = mybir.dt.bfloat16
    f32 = mybir.dt.float32

    nc = bacc.Bacc(
        "TRN2",
        target_bir_lowering=False,
        debug=False,
        num_devices=E,
    )

    # Packed DRAM inputs (per core = one expert):
    # xt:  [NT, P, KD, TB]   xT tiles, d = k*128 + p
    # gw:  [NIC, P, KD, IC]  gate tiles
    # uw:  [NIC, P, KD, IC]  inner tiles
    # ow:  [NDC, P, KI, DC]  output-proj tiles
    xt_d = nc.dram_tensor("xt", (NT, P, KD, TB), bf16, kind="ExternalInput")
    gw_d = nc.dram_tensor("gw", (NIC, P, KD, IC), bf16, kind="ExternalInput")
    uw_d = nc.dram_tensor("uw", (NIC, P, KD, IC), bf16, kind="ExternalInput")
    ow_d = nc.dram_tensor("ow", (NDC, P, KI, DC), bf16, kind="ExternalInput")
    out_d = nc.dram_tensor("out", (T, D), f32, kind="ExternalOutput")

    xt_ap = xt_d.ap()
    gw_ap = gw_d.ap()
    uw_ap = uw_d.ap()
    ow_ap = ow_d.ap()
    # out rows = tb*128 + p
    out_ap = out_d.ap().rearrange("(tb p) d -> tb p d", p=P)

    MI = IC // P  # 2 mi groups per I chunk

    with tile.TileContext(nc) as tc:
        with (
            tc.tile_pool(name="xt", bufs=2) as xt_pool,
            tc.tile_pool(name="w", bufs=2) as w_pool,
            tc.tile_pool(name="ow", bufs=(3 if deep_bufs else 2)) as ow_pool,
            tc.tile_pool(name="ht", bufs=2) as ht_pool,
            tc.tile_pool(name="tmp", bufs=(3 if deep_bufs else 2)) as tmp_pool,
            tc.tile_pool(name="osb", bufs=(3 if deep_bufs else 2)) as osb_pool,
            tc.tile_pool(name="pg", bufs=(8 if psum_shared else 2),
                         space="PSUM") as pg_pool,
            tc.tile_pool(name="pu", bufs=2, space="PSUM") as pu_pool_,
            tc.tile_pool(name="po", bufs=2, space="PSUM") as po_pool_,
        ):
          pu_pool = pg_pool if psum_shared else pu_pool_
          po_pool = pg_pool if psum_shared else po_pool_
          ptag = ("ps", "ps", "ps") if psum_shared else ("pg", "pu", "po")
          for _rep in range(reps):
            for pair in range(NPAIR):
                xts = []
                for j in range(2):
                    xt = xt_pool.tile([P, KD, TB], bf16, tag="xt")
                    (nc.scalar if qsplit else nc.sync).dma_start(
                        xt[:], xt_ap[2 * pair + j])
                    xts.append(xt)

                hts = [
                    ht_pool.tile([P, KI, TB], bf16, tag="ht", name=f"ht{j}")
                    for j in range(2)
                ]

                for ic in range(NIC):
                    gw = w_pool.tile([P, KD, IC], bf16, tag="gw")
                    uw = w_pool.tile([P, KD, IC], bf16, tag="uw")
                    nc.sync.dma_start(gw[:], gw_ap[ic])
                    (nc.scalar if qsplit else nc.sync).dma_start(uw[:], uw_ap[ic])

                    for j in range(2):
                        xt = xts[j]
                        ht = hts[j]
                        for mi in range(MI):
                            pg = pg_pool.tile([P, TB], f32, tag=ptag[0], name="pg")
                            for k in range(KD):
                                nc.tensor.matmul(
                                    pg[:],
                                    gw[:, k, mi * P:(mi + 1) * P],
                                    xt[:, k, :],
                                    start=(k == 0),
                                    stop=(k == KD - 1),
                                )
                            pu = pu_pool.tile([P, TB], f32, tag=ptag[1], name="pu")
                            for k in range(KD):
                                nc.tensor.matmul(
                                    pu[:],
                                    uw[:, k, mi * P:(mi + 1) * P],
                                    xt[:, k, :],
                                    start=(k == 0),
                                    stop=(k == KD - 1),
                                )
                            tmp = tmp_pool.tile([P, TB], f32, tag="tmp")
                            nc.scalar.activation(
                                tmp[:], pg[:], mybir.ActivationFunctionType.Silu
                            )
                            nc.vector.tensor_tensor(
                                ht[:, ic * MI + mi, :],
                                tmp[:],
                                pu[:],
                                mybir.AluOpType.mult,
                            )

                for dci in range(NDC):
                    ow = ow_pool.tile([P, KI, DC], bf16, tag="ow")
                    ((nc.scalar if dci % 2 else nc.sync) if qsplit
                     else nc.sync).dma_start(ow[:], ow_ap[dci])
                    for j in range(2):
                        ht = hts[j]
                        for ti in range(TB // P):
                            po = po_pool.tile([P, DC], f32, tag=ptag[2], name="po")
                            for k in range(KI):
                                nc.tensor.matmul(
                                    po[:],
                                    ht[:, k, ti * P:(ti + 1) * P],
                                    ow[:, k, :],
                                    start=(k == 0),
                                    stop=(k == KI - 1),
                                )
                            osb = osb_pool.tile([P, DC], f32, tag="osb")
                            nc.vector.tensor_copy(osb[:], po[:])
                            nc.sync.dma_start(
                                out_ap[
                                    (2 * pair + j) * (TB // P) + ti,
                                    :,
                                    dci * DC:(dci + 1) * DC,
                                ],
                                osb[:],
                            )

    nc.compile()
    return nc


def _get_program():
    global _COMPILED
    if _COMPILED is None:
        _COMPILED = _build_program(psum_shared=True)
    return _COMPILED


def _pack_inputs(x, gate_proj, inner_proj, output_proj):
    bf16 = ml_dtypes.bfloat16
    in_maps = []
    for e in range(E):
        # xT [D, T] -> [NT, P, KD, TB]; d = k*P + p
        xt = np.ascontiguousarray(x[e].T).astype(bf16)
        xt = xt.reshape(KD, P, NT, TB).transpose(2, 1, 0, 3)
        xt = np.ascontiguousarray(xt)
        # gate/inner [D, I] -> [NIC, P, KD, IC]
        gw = gate_proj[e].astype(bf16).reshape(KD, P, NIC, IC).transpose(2, 1, 0, 3)
        gw = np.ascontiguousarray(gw)
        uw = inner_proj[e].astype(bf16).reshape(KD, P, NIC, IC).transpose(2, 1, 0, 3)
        uw = np.ascontiguousarray(uw)
        # outp [I, D] -> [NDC, P, KI, DC]
        ow = output_proj[e].astype(bf16).reshape(KI, P, NDC, DC).transpose(2, 1, 0, 3)
        ow = np.ascontiguousarray(ow)
        in_maps.append({"xt": xt, "gw": gw, "uw": uw, "ow": ow})
    return in_maps


def kernel(x, gate_proj, inner_proj, output_proj, _trace=False, _trace_kwargs=None):
    from concourse import bass_utils

    nc = _get_program()
    in_maps = _pack_inputs(
        np.asarray(x), np.asarray(gate_proj), np.asarray(inner_proj),
        np.asarray(output_proj),
    )
    res = bass_utils.run_bass_kernel_spmd(
        nc,
        in_maps,
        core_ids=list(range(E)),
        trace=_trace,
        **(_trace_kwargs or {}),
    )
    out = np.stack([np.asarray(res.results[e]["out"]) for e in range(E)])
    out = out.astype(np.float32, copy=False)
    if _trace:
        return out, res
    return out



# revision 2
# speedup vs baseline: 1.0197x; 1.0197x over previous
"""GroupedExpertNetwork (SwiGLU per-expert MLP) Trainium2 kernel, v2.

Expert-parallel: 8 experts -> 8 NeuronCores, one expert per core.
Per core:  g = x @ gate; u = x @ inner; h = silu(g)*u; out = h @ outp
Shapes per expert: x [T=2048, D=2048], gate/inner [D, I=4096], outp [I, D=2048].

Single-pass structure (vs v1's paired T blocks):
  Phase 1: for each of 32 i-chunks, accumulate g and u over the full
  T=2048 moving dim (4 x N=512 matmuls per 128-contraction chunk into a
  4-bank PSUM group), weights stationary. hT [i, t] stays fully
  SBUF-resident (128 KB/partition).
  Phase 2: out^T [d, t] per 128-row d-block, contraction over I with hT
  chunks as the moving operand, ow chunks stationary, N=512 free dim.

Every weight byte is streamed from HBM exactly once: ~67 MB/core total
DMA vs v1's 126 MB. Output is written as bf16 out^T and transposed /
upcast on host. Weight tiles are 2 KB quarter-tiles in a 4-slot pool so
db0's output weights prefetch through the same pool during late phase 1
(the phase-2 ow pool reuses phase-1 SBUF addresses, which carries an
anti-dependency on all phase-1 matmuls - prefetching through the w pool
sidesteps that). Out-DMA rides the scalar queue; the gpsimd software-DGE
path is ~80x slower. All matmul operands bf16, f32 PSUM accumulation.
"""

import numpy as np
import ml_dtypes

E, T, D, I = 8, 2048, 2048, 4096
P = 128
KD = D // P    # 16 contraction chunks (gate/inner)
NIC = I // P   # 32 i-chunks
KI = I // P    # 32 contraction chunks (output)
NDB = D // P   # 16 output d-blocks
KH = 8          # (unused legacy constant, kept for layout notes)
CH = 512       # matmul free-dim chunk (one PSUM bank)
DCH = 1024     # ACT/DVE drain chunk

_COMPILED = None


def _build_program():
    import concourse.mybir as mybir
    import concourse.tile as tile
    from concourse import bacc

    bf16 = mybir.dt.bfloat16
    f32 = mybir.dt.float32

    nc = bacc.Bacc(
        "TRN2",
        target_bir_lowering=False,
        debug=False,
        num_devices=E,
    )

    # Packed DRAM inputs (per core = one expert):
    # xt:  [KD, P, T]        xT tiles, d = k*P + p
    # gw:  [NIC, P, KD, P]   gate tiles, (ic,p,k,i) = gate[k*P+p, ic*P+i]
    # uw:  [NIC, P, KD, P]   inner tiles
    # ow:  [NDB, P, KI, P]   outp tiles, (db,p,k,dd) = outp[k*P+p, db*P+dd]
    # out: [NDB, P, T]       out^T tiles, (db,p,t) = out[t, db*P+p]
    xt_d = nc.dram_tensor("xt", (KD, P, T), bf16, kind="ExternalInput")
    gw_d = nc.dram_tensor("gw", (NIC, P, KD, P), bf16, kind="ExternalInput")
    uw_d = nc.dram_tensor("uw", (NIC, P, KD, P), bf16, kind="ExternalInput")
    ow_d = nc.dram_tensor("ow", (NDB, P, KI, P), bf16, kind="ExternalInput")
    out_d = nc.dram_tensor("out", (NDB, P, T), bf16, kind="ExternalOutput")

    xt_ap = xt_d.ap()
    gw_ap = gw_d.ap()
    uw_ap = uw_d.ap()
    ow_ap = ow_d.ap()
    out_ap = out_d.ap()

    NCH = T // CH

    with tile.TileContext(nc) as tc:
        with (
            tc.tile_pool(name="ht", bufs=1) as ht_pool,
            tc.tile_pool(name="ps", bufs=2, space="PSUM") as ps_pool,
        ):
            ht = ht_pool.tile([P, KI, T], bf16, tag="ht")

            with (
                tc.tile_pool(name="xt", bufs=KD) as xt_pool,
                tc.tile_pool(name="w", bufs=2) as w_pool,
                tc.tile_pool(name="tmp", bufs=3) as tmp_pool,
            ):
                # 16 separate x tiles: per-tile DMA deps, so matmul k only
                # waits for its own 0.5 MB slice instead of all 8.4 MB.
                # Queue order tuned so each tile lands just ahead of its
                # first consumer (only sync + scalar have fast HW DMA
                # queues on TRN2).
                xts = [
                    xt_pool.tile([P, T], bf16, tag="xt", name=f"xt{k}")
                    for k in range(KD)
                ]
                gwt = w_pool.tile([P, KD, P], bf16, tag="w", name="gwt")
                nc.sync.dma_start(gwt[:], gw_ap[0])
                for k in range(0, KD, 2):
                    nc.sync.dma_start(xts[k][:], xt_ap[k])
                for k in (1, 3, 5):
                    nc.scalar.dma_start(xts[k][:], xt_ap[k])
                uwt = w_pool.tile([P, KD, P], bf16, tag="w", name="uwt")
                nc.scalar.dma_start(uwt[:], uw_ap[0])
                for k in (7, 9, 11, 13, 15):
                    nc.scalar.dma_start(xts[k][:], xt_ap[k])

                for ic in range(NIC):
                    if ic > 0:
                        gwt = w_pool.tile([P, KD, P], bf16, tag="w", name="gwt")
                        nc.sync.dma_start(gwt[:], gw_ap[ic])
                        uwt = w_pool.tile([P, KD, P], bf16, tag="w", name="uwt")
                        nc.scalar.dma_start(uwt[:], uw_ap[ic])

                    pg = ps_pool.tile([P, T], f32, tag="ps", name="pg")
                    for k in range(KD):
                        for c in range(NCH):
                            nc.tensor.matmul(
                                pg[:, c * CH:(c + 1) * CH],
                                gwt[:, k, :],
                                xts[k][:, c * CH:(c + 1) * CH],
                                start=(k == 0),
                                stop=(k == KD - 1),
                            )
                    pu = ps_pool.tile([P, T], f32, tag="ps", name="pu")
                    for k in range(KD):
                        for c in range(NCH):
                            nc.tensor.matmul(
                                pu[:, c * CH:(c + 1) * CH],
                                uwt[:, k, :],
                                xts[k][:, c * CH:(c + 1) * CH],
                                start=(k == 0),
                                stop=(k == KD - 1),
                            )
                    for c in range(T // DCH):
                        tmp = tmp_pool.tile([P, DCH], bf16, tag="tmp")
                        nc.scalar.activation(
                            tmp[:],
                            pg[:, c * DCH:(c + 1) * DCH],
                            mybir.ActivationFunctionType.Silu,
                        )
                        nc.vector.tensor_tensor(
                            ht[:, ic, c * DCH:(c + 1) * DCH],
                            tmp[:],
                            pu[:, c * DCH:(c + 1) * DCH],
                            mybir.AluOpType.mult,
                        )

                # Prefetch db0's output weights through the w pool: these
                # slots' anti-deps are on ic31's matmuls, so the DMAs land
                # by the end of phase 1 and phase 2 starts with no PE gap.
                # (The phase-2 ow pool below reuses phase-1 SBUF addresses,
                # carrying an anti-dep on ALL phase-1 matmuls - its first
                # DMA cannot land early.)
                ow0a = w_pool.tile([P, KI // 2, P], bf16, tag="w", name="ow0a")
                nc.sync.dma_start(ow0a[:], ow_ap[0, :, 0:KI // 2])
                ow0b = w_pool.tile([P, KI // 2, P], bf16, tag="w", name="ow0b")
                nc.sync.dma_start(ow0b[:], ow_ap[0, :, KI // 2:KI])

            with (
                tc.tile_pool(name="ow", bufs=2) as ow_pool,
                tc.tile_pool(name="osb", bufs=2) as osb_pool,
            ):
                for db in range(NDB):
                    if db == 0:
                        owts = [ow0a, ow0b]
                        qh = KI // 2
                    else:
                        owt = ow_pool.tile([P, KI, P], bf16, tag="ow",
                                           name="owt")
                        nc.sync.dma_start(owt[:], ow_ap[db])
                        owts = [owt]
                        qh = KI
                    po = ps_pool.tile([P, T], f32, tag="ps", name="po")
                    for k in range(KI):
                        w = owts[k // qh]
                        for c in range(NCH):
                            nc.tensor.matmul(
                                po[:, c * CH:(c + 1) * CH],
                                w[:, k % qh, :],
                                ht[:, k, c * CH:(c + 1) * CH],
                                start=(k == 0),
                                stop=(k == KI - 1),
                            )
                    osb = osb_pool.tile([P, T], bf16, tag="osb")
                    # Last d-block: fine-grained drain on two queues so the
                    # kernel tail is short.
                    dch = CH if db == NDB - 1 else DCH
                    for c in range(T // dch):
                        nc.vector.tensor_copy(
                            osb[:, c * dch:(c + 1) * dch],
                            po[:, c * dch:(c + 1) * dch],
                        )
                        q = nc.scalar if c % 2 == 0 else nc.sync
                        q.dma_start(
                            out_ap[db, :, c * dch:(c + 1) * dch],
                            osb[:, c * dch:(c + 1) * dch],
                        )

    nc.compile()
    return nc


def _get_program():
    global _COMPILED
    if _COMPILED is None:
        _COMPILED = _build_program()
    return _COMPILED


def _pack_inputs(x, gate_proj, inner_proj, output_proj):
    bf16 = ml_dtypes.bfloat16
    in_maps = []
    for e in range(E):
        # xT [D, T] -> [KD, P, T]
        xt = np.ascontiguousarray(x[e].T).astype(bf16).reshape(KD, P, T)
        # gate/inner [D, I] -> [NIC, P, KD, P]
        gw = gate_proj[e].astype(bf16).reshape(KD, P, NIC, P).transpose(2, 1, 0, 3)
        gw = np.ascontiguousarray(gw)
        uw = inner_proj[e].astype(bf16).reshape(KD, P, NIC, P).transpose(2, 1, 0, 3)
        uw = np.ascontiguousarray(uw)
        # outp [I, D] -> [NDB, P, KI, P]
        ow = output_proj[e].astype(bf16).reshape(KI, P, NDB, P).transpose(2, 1, 0, 3)
        ow = np.ascontiguousarray(ow)
        in_maps.append({"xt": xt, "gw": gw, "uw": uw, "ow": ow})
    return in_maps


def kernel(x, gate_proj, inner_proj, output_proj, _trace=False, _trace_kwargs=None):
    from concourse import bass_utils

    nc = _get_program()
    in_maps = _pack_inputs(
        np.asarray(x), np.asarray(gate_proj), np.asarray(inner_proj),
        np.asarray(output_proj),
    )
    res = bass_utils.run_bass_kernel_spmd(
        nc,
        in_maps,
        core_ids=list(range(E)),
        trace=_trace,
        **(_trace_kwargs or {}),
    )
    # out dram [NDB, P, T] = out^T; transpose back to [T, D] and upcast.
    out = np.stack(
        [
            np.asarray(res.results[e]["out"])
            .reshape(D, T)
            .T.astype(np.float32)
            for e in range(E)
        ]
    )
    if _trace:
        return out, res
    return out
